# revision 18
# baseline (speedup 1.0000x reference)
"""Trainium2 Bass kernel for nn_CellularAutomatonDecoder.

Model (per reference):
  cells = embed[tokens] + pos_embed                        (B, T, D)
  rule_bias MLP from mean(c_states); const_bias = rule_bias @ W1b + b1
  8x CA steps: pre = cells@W1c + roll(cells,+1)@W1l + roll(cells,-1)@W1r + const_bias
               cells = a*cells + (1-a)*tanh(gelu(pre) @ W2 + b2)
  out = LN(cells) @ head_w                                 (B, T, V)

Sharding: pure data-parallel over batch across 8 cores (256 rows each).

Device design notes (v3.1, all-bf16 + halo):
- feature-major state sigma[d=128, 8192] bf16 with a 256-col halo on both
  sides, token order t-major (col j = t*256 + b_local): the T-axis roll is
  a +-256 column shift; the halo (refreshed by two small DVE copies per
  step) makes every tap matmul a single contiguous N=512 stream.
- bf16 matmuls everywhere (216ns/512-col vs 230ns fp32r, half the
  SBUF/LDWEIGHTS traffic); state kept scaled sigma = cells/(1-a) so the
  leaky blend is one DVE STT (bf16 state, f32r tanh operand - the fastest
  STT dtype mix measured).
- LN stats token-major directly: per 128-token block, sum and sum-of-
  squares are N=1 matmuls with the sigma/sigma^2 block as the stationary
  operand and a ones column moving - no PE micro-transposes, no row
  staging; the inv-std column math runs per chunk right behind the last
  blend so the head never stalls on it.
- head uses sigma blocks as stationary so output lands token-major in
  PSUM; per-token inv-std applied by DVE/Scalar split; output DMAs
  round-robin across four queues; input packs ride parallel queues too.
"""

import os
import sys

import numpy as np

for _p in ("/opt/trn_rl_repo", "/root/.axon_site/_ro/trn_rl_repo"):
    if os.path.isdir(_p) and _p not in sys.path:
        sys.path.append(_p)

from contextlib import ExitStack

import ml_dtypes

import concourse.bacc as bacc
import concourse.tile as tile
from concourse import mybir
from concourse.bass_utils import run_bass_kernel_spmd

F32 = mybir.dt.float32
F32R = mybir.dt.float32r
BF16 = mybir.dt.bfloat16
AF = mybir.ActivationFunctionType
ALU = mybir.AluOpType
AX = mybir.AxisListType

NP_BF16 = ml_dtypes.bfloat16

B, T, D, V, CDIM = 2048, 32, 128, 256, 128
NEV = 8
EPS = 1e-5
NC = 8
BL = B // NC          # 256 batch rows per core
NTOK = BL * T         # 8192 tokens per core
CH = 1024             # token chunk (columns)
NCH = NTOK // CH      # 8 chunks
NBLK = NTOK // 128    # 64 head blocks
HALO = 256

TRACE = False         # test harness may flip this (with prof shim installed)
_CACHE = {}


def _build(a, has_lnb):
    ia = 1.0 - a
    nc = bacc.Bacc("TRN2", target_bir_lowering=False, debug=False, num_devices=NC)

    tok_d = nc.dram_tensor("tok", [1, NTOK], BF16, kind="ExternalInput").ap()
    bpack_d = nc.dram_tensor("bpack", [128, 4], BF16, kind="ExternalInput").ap()
    cpack_d = nc.dram_tensor("cpack", [128, 46], F32, kind="ExternalInput").ap()
    epack_d = nc.dram_tensor("epack", [128, 256], BF16, kind="ExternalInput").ap()
    wpack_d = nc.dram_tensor("wpack", [128, 1280], BF16, kind="ExternalInput").ap()
    fpack_d = nc.dram_tensor("fpack", [128, 768], F32, kind="ExternalInput").ap()
    out_d = nc.dram_tensor("out", [NTOK, V], BF16, kind="ExternalOutput").ap()
    out_r = out_d.rearrange("(b t) v -> b t v", t=T)

    with tile.TileContext(nc) as tc, ExitStack() as ctx:
        # ---- persistent SBUF (inputs ride parallel DMA queues) ----
        wpool = ctx.enter_context(tc.tile_pool(name="weights", bufs=1))
        bpack = wpool.tile([128, 4], BF16, tag="bpack")
        nc.scalar.dma_start(bpack[:], bpack_d)
        cpack = wpool.tile([128, 46], F32, tag="cpack")
        nc.scalar.dma_start(cpack[:], cpack_d)
        epack = wpool.tile([128, 256], BF16, tag="epack")
        nc.scalar.dma_start(epack[:], epack_d)
        wpack = wpool.tile([128, 1280], BF16, tag="wpack")
        nc.sync.dma_start(wpack[:], wpack_d)
        fpack = wpool.tile([128, 768], F32, tag="fpack")
        nc.sync.dma_start(fpack[:], fpack_d)

        ones_s = bpack[:, 0:1]
        emb_s = epack[:, 0:256]
        wc_s, wl_s, wr_s = wpack[:, 0:256], wpack[:, 256:512], wpack[:, 512:768]
        w2_s, hwc_s = wpack[:, 768:1024], wpack[:, 1024:1280]
        w1b_s, wc1_s, wc2_s = fpack[:, 0:256], fpack[:, 256:512], fpack[:, 512:768]
        posT_s, cT_s = cpack[:, 0:32], cpack[:, 32:36]
        bc1_s, bc2_s = cpack[:, 36:38], cpack[:, 38:39]
        b1_s, b2_s = cpack[:, 39:41], cpack[:, 41:42]
        vid_s = cpack[:, 42:44]

        spool = ctx.enter_context(tc.tile_pool(name="state", bufs=1))
        sigh = spool.tile([128, NTOK + 2 * HALO], BF16, tag="sigma")
        sig = sigh[:, HALO:HALO + NTOK]
        stats_tm = spool.tile([128, 2 * NBLK], F32, tag="stats_tm")

        mlp_sb = ctx.enter_context(tc.tile_pool(name="mlp_sb", bufs=1))
        cbias_s = mlp_sb.tile([128, 2], F32, tag="cbias")

        # shared pools, all phases (no release barriers)
        pp = ctx.enter_context(tc.tile_pool(name="psum", bufs=1, space="PSUM"))
        sbh = ctx.enter_context(tc.tile_pool(name="h_sb", bufs=4))
        sbt = ctx.enter_context(tc.tile_pool(name="t_sb", bufs=NCH + 1))
        sbtok = ctx.enter_context(tc.tile_pool(name="tok_sb", bufs=2))
        sbst = ctx.enter_context(tc.tile_pool(name="stat_sb", bufs=1))
        sbo = ctx.enter_context(tc.tile_pool(name="out_sb", bufs=4))

        def ptile(shape, tag, name):
            return pp.tile(shape, F32, tag=tag, name=name, bufs=3 if tag == "pre" else 1)

        def emit_halo(ci):
            if ci == 0:
                nc.vector.tensor_copy(sigh[:, HALO + NTOK:], sig[:, 0:HALO])
            if ci == NCH - 1:
                nc.vector.tensor_copy(sigh[:, 0:HALO], sig[:, NTOK - HALO:NTOK])

        # ---- init: token gather via one-hot matmuls ----
        # tokens land on one partition (16KB DMA) and are broadcast on-chip
        # by K=1 ones-matmuls on the (otherwise cold) PE
        tokrow = sbtok.tile([1, NTOK], BF16, tag="tok", name="tokrow")
        nc.gpsimd.dma_start(tokrow[:], tok_d)
        onesrow = sbtok.tile([1, 128], BF16, tag="onesrow", name="onesrow")
        nc.vector.memset(onesrow[:], 1.0)
        for ci in [6, 7, 0, 1, 2, 3, 4, 5]:
            c0 = ci * CH
            tok_ps = ptile([128, CH], "pre", "tok_ps")
            for k in range(2):
                nc.tensor.matmul(tok_ps[:, k * 512:(k + 1) * 512], onesrow[:],
                                 tokrow[0:1, c0 + k * 512:c0 + (k + 1) * 512],
                                 start=True, stop=True)
            oh_lo = sbh.tile([128, CH], BF16, tag="h", name="oh_lo")
            oh_hi = sbh.tile([128, CH], BF16, tag="h", name="oh_hi")
            nc.vector.tensor_scalar(oh_lo[:], tok_ps[:], vid_s[:, 0:1], None,
                                    ALU.is_equal)
            nc.vector.tensor_scalar(oh_hi[:], tok_ps[:], vid_s[:, 1:2], None,
                                    ALU.is_equal)
            cells_ps = ptile([128, CH], "pre", "cells_ps")
            for k in range(2):
                jc = slice(k * 512, (k + 1) * 512)
                nc.tensor.matmul(cells_ps[:, jc], emb_s[:, 0:128], oh_lo[:, jc],
                                 start=True, stop=False)
                nc.tensor.matmul(cells_ps[:, jc], emb_s[:, 128:256], oh_hi[:, jc],
                                 start=False, stop=True)
            for kb in range(CH // 256):
                tt = (c0 + kb * 256) // 256  # col j = t*256 + b -> t = j//256
                pb = posT_s[:, tt:tt + 1].rearrange(
                    "p (o n) -> p o n", o=1).broadcast_to((128, 1, 256))
                nc.vector.tensor_tensor(
                    sig[:, c0 + kb * 256: c0 + (kb + 1) * 256].rearrange(
                        "p (o n) -> p o n", o=1),
                    cells_ps[:, kb * 256:(kb + 1) * 256].rearrange(
                        "p (o n) -> p o n", o=1),
                    pb, op=ALU.add)
            emit_halo(ci)

        # ---- rule-bias MLP (tiny; overlaps gather) ----
        cp_s = mlp_sb.tile([128, 1], F32, tag="cp")
        nc.vector.tensor_reduce(cp_s[:], cT_s[:], axis=AX.X, op=ALU.add)
        y1_ps = ptile([128, 2], "new", "y1_ps")
        for h in range(2):
            nc.tensor.matmul(y1_ps[:, h:h + 1], wc1_s[:, h * 128:(h + 1) * 128],
                             cp_s[:], start=True, stop=True)
        y1g_s = mlp_sb.tile([128, 2], F32, tag="y1g")
        for h in range(2):
            nc.scalar.activation(y1g_s[:, h:h + 1], y1_ps[:, h:h + 1], AF.Gelu,
                                 bias=bc1_s[:, h:h + 1], scale=0.25)
        rb_ps = ptile([128, 2], "new", "rb_ps")
        nc.tensor.matmul(rb_ps[:, 0:1], wc2_s[:, 0:128], y1g_s[:, 0:1],
                         start=True, stop=False)
        nc.tensor.matmul(rb_ps[:, 0:1], wc2_s[:, 128:256], y1g_s[:, 1:2],
                         start=False, stop=True)
        rb_s = mlp_sb.tile([128, 1], F32, tag="rb")
        nc.scalar.activation(rb_s[:], rb_ps[:, 0:1], AF.Identity, bias=bc2_s[:, 0:1])
        cb_ps = ptile([128, 2], "new", "cb_ps")
        for h in range(2):
            nc.tensor.matmul(cb_ps[:, h:h + 1], w1b_s[:, h * 128:(h + 1) * 128],
                             rb_s[:], start=True, stop=True)
        for h in range(2):
            nc.scalar.activation(cbias_s[:, h:h + 1], cb_ps[:, h:h + 1], AF.Identity,
                                 bias=b1_s[:, h:h + 1])

        # ---- evolve: 8 CA steps ----
        def emit_chunk(ci):
            c0 = ci * CH
            pre = [ptile([128, CH], "pre", f"pre{h_}") for h_ in range(2)]
            for h in range(2):
                hcols = slice(h * 128, (h + 1) * 128)
                for k in range(2):
                    j0 = HALO + c0 + k * 512
                    dst = pre[h][:, k * 512:(k + 1) * 512]
                    for i, (w, off) in enumerate(
                            ((wc_s, 0), (wl_s, -HALO), (wr_s, +HALO))):
                        nc.tensor.matmul(dst, w[:, hcols],
                                         sigh[:, j0 + off:j0 + off + 512],
                                         start=(i == 0), stop=(i == 2))
            h_t = [sbh.tile([128, CH], BF16, tag="h", name=f"ht{h_}")
                   for h_ in range(2)]
            for h in range(2):
                nc.scalar.activation(h_t[h][:], pre[h][:], AF.Gelu,
                                     bias=cbias_s[:, h:h + 1], scale=ia)
            new_ps = ptile([128, CH], "new", "new_ps")
            for k in range(2):
                jc = slice(k * 512, (k + 1) * 512)
                nc.tensor.matmul(new_ps[:, jc], w2_s[:, 0:128], h_t[0][:, jc],
                                 start=True, stop=False)
                nc.tensor.matmul(new_ps[:, jc], w2_s[:, 128:256], h_t[1][:, jc],
                                 start=False, stop=True)
            t_t = sbt.tile([128, CH], F32R, tag="t", name="t_t")
            nc.scalar.activation(t_t[:], new_ps[:], AF.Tanh, bias=b2_s[:, 0:1])
            return t_t

        def emit_blend(ci, t_t):
            c0 = ci * CH
            nc.vector.scalar_tensor_tensor(
                sig[:, c0:c0 + CH], sig[:, c0:c0 + CH], a, t_t[:],
                op0=ALU.mult, op1=ALU.add)
            emit_halo(ci)

        st3 = stats_tm[:].rearrange("p (b two) -> p b two", two=2)
        m2_s = sbst.tile([128, NBLK], F32, tag="m2")
        vf_s = sbst.tile([128, NBLK], F32, tag="vf")
        sd_s = sbst.tile([128, NBLK], F32, tag="sd")
        y0_s = sbst.tile([128, NBLK], F32, tag="y0")
        q_s = sbst.tile([128, NBLK], F32, tag="q")
        w_s = sbst.tile([128, NBLK], F32, tag="w")
        inv_s = sbst.tile([128, NBLK], F32, tag="inv")

        def emit_stats(ci):
            c0 = ci * CH
            sq_t = sbh.tile([128, CH], BF16, tag="h", name="sq_t")
            nc.vector.tensor_mul(sq_t[:], sig[:, c0:c0 + CH], sig[:, c0:c0 + CH])
            st_ps = ptile([128, 16], "new", "st_ps")
            for j in range(CH // 128):
                nc.tensor.matmul(st_ps[:, 2 * j:2 * j + 1],
                                 sig[:, c0 + j * 128:c0 + (j + 1) * 128],
                                 ones_s, start=True, stop=True)
                nc.tensor.matmul(st_ps[:, 2 * j + 1:2 * j + 2],
                                 sq_t[:, j * 128:(j + 1) * 128],
                                 ones_s, start=True, stop=True)
            nc.vector.tensor_copy(stats_tm[:, 16 * ci:16 * (ci + 1)], st_ps[:])
            # per-chunk inv-std column math (token-major [128, 8] slices)
            sl = slice(8 * ci, 8 * (ci + 1))
            nc.vector.scalar_tensor_tensor(m2_s[:, sl], st3[:, sl, 0],
                                           (ia / 128.0) ** 2, st3[:, sl, 0],
                                           op0=ALU.mult, op1=ALU.mult)
            nc.vector.scalar_tensor_tensor(vf_s[:, sl], st3[:, sl, 1],
                                           ia * ia / 128.0, m2_s[:, sl],
                                           op0=ALU.mult, op1=ALU.subtract)
            nc.vector.tensor_scalar_add(vf_s[:, sl], vf_s[:, sl], EPS)
            nc.scalar.activation(sd_s[:, sl], vf_s[:, sl], AF.Sqrt)
            nc.vector.reciprocal(y0_s[:, sl], sd_s[:, sl])
            nc.vector.tensor_mul(q_s[:, sl], y0_s[:, sl], y0_s[:, sl])
            nc.vector.scalar_tensor_tensor(w_s[:, sl], vf_s[:, sl], -0.5,
                                           q_s[:, sl], op0=ALU.mult, op1=ALU.mult)
            nc.vector.scalar_tensor_tensor(inv_s[:, sl], w_s[:, sl], 1.5,
                                           y0_s[:, sl], op0=ALU.add, op1=ALU.mult)

        for s in range(NEV - 1):
            order = [(s + j) % NCH for j in range(NCH)]
            t_tiles = {}
            for i, ci in enumerate(order):
                t_tiles[ci] = emit_chunk(ci)
                if i >= 2:
                    emit_blend(order[i - 1], t_tiles[order[i - 1]])
            emit_blend(order[NCH - 1], t_tiles[order[NCH - 1]])
            emit_blend(order[0], t_tiles[order[0]])

        # last step: blends lag chunk processing by 2; each chunk's LN stats
        # + inv-std math follow its blend immediately
        P = [(NCH - 2 + j) % NCH for j in range(NCH)]
        t7 = {}
        warm_s = sbst.tile([1, 8], F32, tag="warm")
        nc.scalar.activation(warm_s[:], cpack[0:1, 0:8], AF.Sqrt)
        for i, ci in enumerate(P):
            t7[ci] = emit_chunk(ci)
            if i >= 2:
                emit_blend(P[i - 1], t7[P[i - 1]])
                emit_stats(P[i - 1])
        for ci in (P[7], P[0]):
            emit_blend(ci, t7[ci])
            emit_stats(ci)

        # ---- final: head, chunk-completion order ----
        # out[tok, v] = inv[tok] * (sigma_blk^T @ hwc); two blocks with
        # consecutive t at a fixed batch-half share one 128KB bf16 DMA whose
        # per-partition run is 1KB contiguous (3D access pattern)
        dmaq = [nc.sync, nc.scalar, nc.gpsimd]
        heads = [(ci, hh) for ci in P for hh in range(2)]
        n_dma = 0
        for ci, hh in heads:
            for half in range(2):
                t0g = ci * 4 + half * 2
                b0 = hh * 128
                o_t = sbo.tile([128, 2 * V], BF16, tag="o", name="o_t")
                for tl in range(2):
                    b = 2 * (t0g + tl) + hh
                    A_ps = ptile([128, V], "pre", "A_ps")
                    nc.tensor.matmul(A_ps[:], sig[:, b * 128:(b + 1) * 128],
                                     hwc_s[:], start=True, stop=True)
                    nc.vector.tensor_scalar(o_t[:, tl * V:(tl + 1) * V], A_ps[:],
                                            inv_s[:, b:b + 1], None, ALU.mult)
                dmaq[n_dma % 3].dma_start(
                    out_r[b0:b0 + 128, t0g:t0g + 2, :],
                    o_t[:].rearrange("p (t v) -> p t v", t=2))
                n_dma += 1

    nc.compile()
    return nc


def kernel(**inputs):
    g = {k: np.asarray(v, np.float32) if k != "tokens" else np.asarray(v)
         for k, v in inputs.items()}
    alpha = float(g["alpha"])
    a = float(1.0 / (1.0 + np.exp(-np.float64(alpha))))
    ia = 1.0 - a
    ln_b = g["ln_b"]
    has_lnb = bool(np.any(ln_b != 0))
    key = (np.float64(a).tobytes(), has_lnb)
    if key not in _CACHE:
        _CACHE[key] = _build(a, has_lnb)
    nc = _CACHE[key]

    W1, W2 = g["W1"], g["W2"]
    embed, pos = g["embed"], g["pos_embed"]
    head_w, ln_g = g["head_w"], g["ln_g"]

    bpack = np.zeros((128, 4), np.float32)
    bpack[:, 0] = 1.0
    bpack = bpack.astype(NP_BF16)

    cpack = np.zeros((128, 46), np.float32)
    cpack[:, 0:32] = pos.T * np.float32(1.0 / ia)
    cpack[:, 32:36] = g["c_states"].T
    cpack[:, 36:38] = g["bc1"].reshape(2, 128).T
    cpack[:, 38:39] = g["bc2"].reshape(128, 1)
    cpack[:, 39:41] = g["b1"].reshape(2, 128).T
    cpack[:, 41:42] = g["b2"].reshape(128, 1)
    cpack[:, 42:44] = np.stack([np.arange(128), np.arange(128, 256)], axis=1)

    epack = (np.concatenate([embed[0:128], embed[128:256]], axis=1)
             * np.float32(1.0 / ia)).astype(NP_BF16)

    wpack = np.zeros((128, 1280), np.float32)
    wpack[:, 0:256] = W1[0:128]
    wpack[:, 256:512] = W1[128:256]
    wpack[:, 512:768] = W1[256:384]
    wpack[:, 768:1024] = np.concatenate([W2[0:128], W2[128:256]], axis=1)
    ghw = head_w * ln_g[:, None]
    wpack[:, 1024:1280] = (ghw - ghw.mean(axis=0, keepdims=True)) * np.float32(ia)
    wpack = wpack.astype(NP_BF16)

    fpack = np.zeros((128, 768), np.float32)
    fpack[:, 0:256] = W1[384:512]
    fpack[:, 256:512] = g["Wc1"]
    fpack[:, 512:768] = np.concatenate([g["Wc2"][0:128], g["Wc2"][128:256]], axis=1)

    tokens = g["tokens"]
    in_maps = []
    for c in range(NC):
        tk = tokens[c * BL:(c + 1) * BL].astype(np.float32)   # (BL, T)
        in_maps.append({
            "tok": np.ascontiguousarray(tk.T).reshape(1, NTOK).astype(NP_BF16),
            "bpack": bpack, "cpack": cpack, "epack": epack,
            "wpack": wpack, "fpack": fpack,
        })

    kw = {}
    if TRACE:
        kw = dict(trace=True)
    res = run_bass_kernel_spmd(nc, in_maps, core_ids=list(range(NC)), **kw)
    if TRACE and res.exec_time_ns is not None:
        print(f"HW exec time: {res.exec_time_ns} ns")
        kernel.last_exec_ns = res.exec_time_ns
        kernel.last_trace = res.instructions_and_trace
    out = np.stack([np.asarray(res.results[c]["out"], dtype=np.float32)
                    for c in range(NC)], axis=0)
    out = out.reshape(B, T, V)
    if has_lnb:
        out = out + (ln_b @ head_w)[None, None, :]
    return np.ascontiguousarray(out)


# revision 20
# speedup vs baseline: 1.0334x; 1.0334x over previous
"""Trainium2 Bass kernel for nn_CellularAutomatonDecoder.

Model (per reference):
  cells = embed[tokens] + pos_embed                        (B, T, D)
  rule_bias MLP from mean(c_states); const_bias = rule_bias @ W1b + b1
  8x CA steps: pre = cells@W1c + roll(cells,+1)@W1l + roll(cells,-1)@W1r + const_bias
               cells = a*cells + (1-a)*tanh(gelu(pre) @ W2 + b2)
  out = LN(cells) @ head_w                                 (B, T, V)

Sharding: pure data-parallel over batch across 8 cores (256 rows each).

Device design notes (v3.1, all-bf16 + halo):
- feature-major state sigma[d=128, 8192] bf16 with a 256-col halo on both
  sides, token order t-major (col j = t*256 + b_local): the T-axis roll is
  a +-256 column shift; the halo (refreshed by two small DVE copies per
  step) makes every tap matmul a single contiguous N=512 stream.
- bf16 matmuls everywhere (216ns/512-col vs 230ns fp32r, half the
  SBUF/LDWEIGHTS traffic); state kept scaled sigma = cells/(1-a) so the
  leaky blend is one DVE STT (bf16 state, f32r tanh operand - the fastest
  STT dtype mix measured).
- LN stats token-major directly: per 128-token block, sum and sum-of-
  squares are N=1 matmuls with the sigma/sigma^2 block as the stationary
  operand and a ones column moving - no PE micro-transposes, no row
  staging; the inv-std column math runs per chunk right behind the last
  blend so the head never stalls on it.
- head uses sigma blocks as stationary so output lands token-major in
  PSUM; per-token inv-std applied by DVE/Scalar split; output DMAs
  round-robin across four queues; input packs ride parallel queues too.
"""

import os
import sys

import numpy as np

for _p in ("/opt/trn_rl_repo", "/root/.axon_site/_ro/trn_rl_repo"):
    if os.path.isdir(_p) and _p not in sys.path:
        sys.path.append(_p)

from contextlib import ExitStack

import ml_dtypes

import concourse.bacc as bacc
import concourse.tile as tile
from concourse import mybir
from concourse.bass_utils import run_bass_kernel_spmd

F32 = mybir.dt.float32
F32R = mybir.dt.float32r
BF16 = mybir.dt.bfloat16
AF = mybir.ActivationFunctionType
ALU = mybir.AluOpType
AX = mybir.AxisListType

NP_BF16 = ml_dtypes.bfloat16

B, T, D, V, CDIM = 2048, 32, 128, 256, 128
NEV = 8
EPS = 1e-5
NC = 8
BL = B // NC          # 256 batch rows per core
NTOK = BL * T         # 8192 tokens per core
CH = 1024             # token chunk (columns)
NCH = NTOK // CH      # 8 chunks
NBLK = NTOK // 128    # 64 head blocks
HALO = 256

TRACE = False         # test harness may flip this (with prof shim installed)
_CACHE = {}


def _build(a, has_lnb):
    ia = 1.0 - a
    nc = bacc.Bacc("TRN2", target_bir_lowering=False, debug=False, num_devices=NC)

    tok_d = nc.dram_tensor("tok", [1, NTOK], BF16, kind="ExternalInput").ap()
    bpack_d = nc.dram_tensor("bpack", [128, 4], BF16, kind="ExternalInput").ap()
    cpack_d = nc.dram_tensor("cpack", [128, 46], F32, kind="ExternalInput").ap()
    epack_d = nc.dram_tensor("epack", [128, 256], BF16, kind="ExternalInput").ap()
    wpack_d = nc.dram_tensor("wpack", [128, 1280], BF16, kind="ExternalInput").ap()
    fpack_d = nc.dram_tensor("fpack", [128, 768], F32, kind="ExternalInput").ap()
    out_d = nc.dram_tensor("out", [NTOK, V], BF16, kind="ExternalOutput").ap()
    out_r = out_d.rearrange("(b t) v -> b t v", t=T)

    with tile.TileContext(nc) as tc, ExitStack() as ctx:
        # ---- persistent SBUF (inputs ride parallel DMA queues) ----
        wpool = ctx.enter_context(tc.tile_pool(name="weights", bufs=1))
        bpack = wpool.tile([128, 4], BF16, tag="bpack")
        nc.scalar.dma_start(bpack[:], bpack_d)
        cpack = wpool.tile([128, 46], F32, tag="cpack")
        nc.scalar.dma_start(cpack[:], cpack_d)
        epack = wpool.tile([128, 256], BF16, tag="epack")
        nc.scalar.dma_start(epack[:], epack_d)
        wpack = wpool.tile([128, 1280], BF16, tag="wpack")
        nc.sync.dma_start(wpack[:], wpack_d)
        fpack = wpool.tile([128, 768], F32, tag="fpack")
        nc.sync.dma_start(fpack[:], fpack_d)

        ones_s = bpack[:, 0:1]
        emb_s = epack[:, 0:256]
        wc_s, wl_s, wr_s = wpack[:, 0:256], wpack[:, 256:512], wpack[:, 512:768]
        w2_s, hwc_s = wpack[:, 768:1024], wpack[:, 1024:1280]
        w1b_s, wc1_s, wc2_s = fpack[:, 0:256], fpack[:, 256:512], fpack[:, 512:768]
        posT_s, cT_s = cpack[:, 0:32], cpack[:, 32:36]
        bc1_s, bc2_s = cpack[:, 36:38], cpack[:, 38:39]
        b1_s, b2_s = cpack[:, 39:41], cpack[:, 41:42]
        vid_s = cpack[:, 42:44]

        spool = ctx.enter_context(tc.tile_pool(name="state", bufs=1))
        sigh = spool.tile([128, NTOK + 2 * HALO], BF16, tag="sigma")
        sig = sigh[:, HALO:HALO + NTOK]
        stats_tm = spool.tile([128, 2 * NBLK], F32, tag="stats_tm")

        mlp_sb = ctx.enter_context(tc.tile_pool(name="mlp_sb", bufs=1))
        cbias_s = mlp_sb.tile([128, 2], F32, tag="cbias")

        # shared pools, all phases (no release barriers)
        pp = ctx.enter_context(tc.tile_pool(name="psum", bufs=1, space="PSUM"))
        sbh = ctx.enter_context(tc.tile_pool(name="h_sb", bufs=4))
        sbt = ctx.enter_context(tc.tile_pool(name="t_sb", bufs=NCH + 1))
        sbtok = ctx.enter_context(tc.tile_pool(name="tok_sb", bufs=2))
        sbst = ctx.enter_context(tc.tile_pool(name="stat_sb", bufs=1))
        sbo = ctx.enter_context(tc.tile_pool(name="out_sb", bufs=8))

        def ptile(shape, tag, name):
            return pp.tile(shape, F32, tag=tag, name=name, bufs=3 if tag == "pre" else 1)

        def emit_halo(ci):
            if ci == 0:
                nc.vector.tensor_copy(sigh[:, HALO + NTOK:], sig[:, 0:HALO])
            if ci == NCH - 1:
                nc.vector.tensor_copy(sigh[:, 0:HALO], sig[:, NTOK - HALO:NTOK])

        # ---- init: token gather via one-hot matmuls ----
        # tokens land on one partition (16KB DMA) and are broadcast on-chip
        # by K=1 ones-matmuls on the (otherwise cold) PE
        tokrow = sbtok.tile([1, NTOK], BF16, tag="tok", name="tokrow")
        nc.gpsimd.dma_start(tokrow[:], tok_d)
        onesrow = sbtok.tile([1, 128], BF16, tag="onesrow", name="onesrow")
        nc.vector.memset(onesrow[:], 1.0)
        for ci in [6, 7, 0, 1, 2, 3, 4, 5]:
            c0 = ci * CH
            tok_ps = ptile([128, CH], "pre", "tok_ps")
            for k in range(2):
                nc.tensor.matmul(tok_ps[:, k * 512:(k + 1) * 512], onesrow[:],
                                 tokrow[0:1, c0 + k * 512:c0 + (k + 1) * 512],
                                 start=True, stop=True)
            tokb = sbh.tile([128, CH], BF16, tag="h", name="tokb")
            nc.scalar.activation(tokb[:], tok_ps[:], AF.Copy)
            oh_lo = sbh.tile([128, CH], BF16, tag="h", name="oh_lo")
            oh_hi = sbh.tile([128, CH], BF16, tag="h", name="oh_hi")
            nc.vector.tensor_scalar(oh_lo[:], tokb[:], vid_s[:, 0:1], None,
                                    ALU.is_equal)
            nc.vector.tensor_scalar(oh_hi[:], tokb[:], vid_s[:, 1:2], None,
                                    ALU.is_equal)
            cells_ps = ptile([128, CH], "pre", "cells_ps")
            for k in range(2):
                jc = slice(k * 512, (k + 1) * 512)
                nc.tensor.matmul(cells_ps[:, jc], emb_s[:, 0:128], oh_lo[:, jc],
                                 start=True, stop=False)
                nc.tensor.matmul(cells_ps[:, jc], emb_s[:, 128:256], oh_hi[:, jc],
                                 start=False, stop=True)
            for kb in range(CH // 256):
                tt = (c0 + kb * 256) // 256  # col j = t*256 + b -> t = j//256
                pb = posT_s[:, tt:tt + 1].rearrange(
                    "p (o n) -> p o n", o=1).broadcast_to((128, 1, 256))
                nc.vector.tensor_tensor(
                    sig[:, c0 + kb * 256: c0 + (kb + 1) * 256].rearrange(
                        "p (o n) -> p o n", o=1),
                    cells_ps[:, kb * 256:(kb + 1) * 256].rearrange(
                        "p (o n) -> p o n", o=1),
                    pb, op=ALU.add)
            emit_halo(ci)

        # ---- rule-bias MLP (tiny; overlaps gather) ----
        cp_s = mlp_sb.tile([128, 1], F32, tag="cp")
        nc.vector.tensor_reduce(cp_s[:], cT_s[:], axis=AX.X, op=ALU.add)
        y1_ps = ptile([128, 2], "new", "y1_ps")
        for h in range(2):
            nc.tensor.matmul(y1_ps[:, h:h + 1], wc1_s[:, h * 128:(h + 1) * 128],
                             cp_s[:], start=True, stop=True)
        y1g_s = mlp_sb.tile([128, 2], F32, tag="y1g")
        for h in range(2):
            nc.scalar.activation(y1g_s[:, h:h + 1], y1_ps[:, h:h + 1], AF.Gelu,
                                 bias=bc1_s[:, h:h + 1], scale=0.25)
        rb_ps = ptile([128, 2], "new", "rb_ps")
        nc.tensor.matmul(rb_ps[:, 0:1], wc2_s[:, 0:128], y1g_s[:, 0:1],
                         start=True, stop=False)
        nc.tensor.matmul(rb_ps[:, 0:1], wc2_s[:, 128:256], y1g_s[:, 1:2],
                         start=False, stop=True)
        rb_s = mlp_sb.tile([128, 1], F32, tag="rb")
        nc.scalar.activation(rb_s[:], rb_ps[:, 0:1], AF.Identity, bias=bc2_s[:, 0:1])
        cb_ps = ptile([128, 2], "new", "cb_ps")
        for h in range(2):
            nc.tensor.matmul(cb_ps[:, h:h + 1], w1b_s[:, h * 128:(h + 1) * 128],
                             rb_s[:], start=True, stop=True)
        for h in range(2):
            nc.scalar.activation(cbias_s[:, h:h + 1], cb_ps[:, h:h + 1], AF.Identity,
                                 bias=b1_s[:, h:h + 1])

        # ---- evolve: 8 CA steps ----
        def emit_chunk(ci):
            c0 = ci * CH
            pre = [ptile([128, CH], "pre", f"pre{h_}") for h_ in range(2)]
            for h in range(2):
                hcols = slice(h * 128, (h + 1) * 128)
                for k in range(2):
                    j0 = HALO + c0 + k * 512
                    dst = pre[h][:, k * 512:(k + 1) * 512]
                    for i, (w, off) in enumerate(
                            ((wc_s, 0), (wl_s, -HALO), (wr_s, +HALO))):
                        nc.tensor.matmul(dst, w[:, hcols],
                                         sigh[:, j0 + off:j0 + off + 512],
                                         start=(i == 0), stop=(i == 2))
            h_t = [sbh.tile([128, CH], BF16, tag="h", name=f"ht{h_}")
                   for h_ in range(2)]
            for h in range(2):
                nc.scalar.activation(h_t[h][:], pre[h][:], AF.Gelu,
                                     bias=cbias_s[:, h:h + 1], scale=ia)
            new_ps = ptile([128, CH], "new", "new_ps")
            for k in range(2):
                jc = slice(k * 512, (k + 1) * 512)
                nc.tensor.matmul(new_ps[:, jc], w2_s[:, 0:128], h_t[0][:, jc],
                                 start=True, stop=False)
                nc.tensor.matmul(new_ps[:, jc], w2_s[:, 128:256], h_t[1][:, jc],
                                 start=False, stop=True)
            t_t = sbt.tile([128, CH], F32R, tag="t", name="t_t")
            nc.scalar.activation(t_t[:], new_ps[:], AF.Tanh, bias=b2_s[:, 0:1])
            return t_t

        def emit_blend(ci, t_t):
            c0 = ci * CH
            nc.vector.scalar_tensor_tensor(
                sig[:, c0:c0 + CH], sig[:, c0:c0 + CH], a, t_t[:],
                op0=ALU.mult, op1=ALU.add)
            emit_halo(ci)

        st3 = stats_tm[:].rearrange("p (b two) -> p b two", two=2)
        m2_s = sbst.tile([128, NBLK], F32, tag="m2")
        vf_s = sbst.tile([128, NBLK], F32, tag="vf")
        sd_s = sbst.tile([128, NBLK], F32, tag="sd")
        y0_s = sbst.tile([128, NBLK], F32, tag="y0")
        q_s = sbst.tile([128, NBLK], F32, tag="q")
        w_s = sbst.tile([128, NBLK], F32, tag="w")
        inv_s = sbst.tile([128, NBLK], F32, tag="inv")

        def emit_stats(ci):
            c0 = ci * CH
            sq_t = sbh.tile([128, CH], BF16, tag="h", name="sq_t")
            nc.vector.tensor_mul(sq_t[:], sig[:, c0:c0 + CH], sig[:, c0:c0 + CH])
            st_ps = ptile([128, 16], "new", "st_ps")
            for j in range(CH // 128):
                nc.tensor.matmul(st_ps[:, 2 * j:2 * j + 1],
                                 sig[:, c0 + j * 128:c0 + (j + 1) * 128],
                                 ones_s, start=True, stop=True)
                nc.tensor.matmul(st_ps[:, 2 * j + 1:2 * j + 2],
                                 sq_t[:, j * 128:(j + 1) * 128],
                                 ones_s, start=True, stop=True)
            nc.vector.tensor_copy(stats_tm[:, 16 * ci:16 * (ci + 1)], st_ps[:])
            # per-chunk inv-std column math (token-major [128, 8] slices)
            sl = slice(8 * ci, 8 * (ci + 1))
            nc.vector.scalar_tensor_tensor(m2_s[:, sl], st3[:, sl, 0],
                                           (ia / 128.0) ** 2, st3[:, sl, 0],
                                           op0=ALU.mult, op1=ALU.mult)
            nc.vector.scalar_tensor_tensor(vf_s[:, sl], st3[:, sl, 1],
                                           ia * ia / 128.0, m2_s[:, sl],
                                           op0=ALU.mult, op1=ALU.subtract)
            nc.vector.tensor_scalar_add(vf_s[:, sl], vf_s[:, sl], EPS)
            nc.scalar.activation(sd_s[:, sl], vf_s[:, sl], AF.Sqrt)
            nc.vector.reciprocal(y0_s[:, sl], sd_s[:, sl])
            nc.vector.tensor_mul(q_s[:, sl], y0_s[:, sl], y0_s[:, sl])
            nc.vector.scalar_tensor_tensor(w_s[:, sl], vf_s[:, sl], -0.5,
                                           q_s[:, sl], op0=ALU.mult, op1=ALU.mult)
            nc.vector.scalar_tensor_tensor(inv_s[:, sl], w_s[:, sl], 1.5,
                                           y0_s[:, sl], op0=ALU.add, op1=ALU.mult)

        for s in range(NEV - 1):
            order = [(s + j) % NCH for j in range(NCH)]
            t_tiles = {}
            for i, ci in enumerate(order):
                t_tiles[ci] = emit_chunk(ci)
                if i >= 2:
                    emit_blend(order[i - 1], t_tiles[order[i - 1]])
            emit_blend(order[NCH - 1], t_tiles[order[NCH - 1]])
            emit_blend(order[0], t_tiles[order[0]])

        # last step: blends lag chunk processing by 2; each chunk's LN stats
        # + inv-std math follow its blend immediately
        P = [(NCH - 2 + j) % NCH for j in range(NCH)]
        t7 = {}
        warm_s = sbst.tile([1, 8], F32, tag="warm")
        nc.scalar.activation(warm_s[:], cpack[0:1, 0:8], AF.Sqrt)
        for i, ci in enumerate(P):
            t7[ci] = emit_chunk(ci)
            if i >= 2:
                emit_blend(P[i - 1], t7[P[i - 1]])
                emit_stats(P[i - 1])
        for ci in (P[7], P[0]):
            emit_blend(ci, t7[ci])
            emit_stats(ci)

        # ---- final: head, chunk-completion order ----
        # out[tok, v] = inv[tok] * (sigma_blk^T @ hwc); two blocks with
        # consecutive t at a fixed batch-half share one 128KB bf16 DMA whose
        # per-partition run is 1KB contiguous (3D access pattern)
        dmaq = [nc.sync, nc.scalar, nc.gpsimd]
        heads = [(ci, hh) for ci in P for hh in range(2)]
        n_dma = 0
        for ci, hh in heads:
            for half in range(2):
                t0g = ci * 4 + half * 2
                b0 = hh * 128
                o_t = sbo.tile([128, 2 * V], BF16, tag="o", name="o_t")
                for tl in range(2):
                    b = 2 * (t0g + tl) + hh
                    A_ps = ptile([128, V], "pre", "A_ps")
                    nc.tensor.matmul(A_ps[:], sig[:, b * 128:(b + 1) * 128],
                                     hwc_s[:], start=True, stop=True)
                    nc.vector.tensor_scalar(o_t[:, tl * V:(tl + 1) * V], A_ps[:],
                                            inv_s[:, b:b + 1], None, ALU.mult)
                dmaq[n_dma % 3].dma_start(
                    out_r[b0:b0 + 128, t0g:t0g + 2, :],
                    o_t[:].rearrange("p (t v) -> p t v", t=2))
                n_dma += 1

    nc.compile()
    return nc


def kernel(**inputs):
    g = {k: np.asarray(v, np.float32) if k != "tokens" else np.asarray(v)
         for k, v in inputs.items()}
    alpha = float(g["alpha"])
    a = float(1.0 / (1.0 + np.exp(-np.float64(alpha))))
    ia = 1.0 - a
    ln_b = g["ln_b"]
    has_lnb = bool(np.any(ln_b != 0))
    key = (np.float64(a).tobytes(), has_lnb)
    if key not in _CACHE:
        _CACHE[key] = _build(a, has_lnb)
    nc = _CACHE[key]

    W1, W2 = g["W1"], g["W2"]
    embed, pos = g["embed"], g["pos_embed"]
    head_w, ln_g = g["head_w"], g["ln_g"]

    bpack = np.zeros((128, 4), np.float32)
    bpack[:, 0] = 1.0
    bpack = bpack.astype(NP_BF16)

    cpack = np.zeros((128, 46), np.float32)
    cpack[:, 0:32] = pos.T * np.float32(1.0 / ia)
    cpack[:, 32:36] = g["c_states"].T
    cpack[:, 36:38] = g["bc1"].reshape(2, 128).T
    cpack[:, 38:39] = g["bc2"].reshape(128, 1)
    cpack[:, 39:41] = g["b1"].reshape(2, 128).T
    cpack[:, 41:42] = g["b2"].reshape(128, 1)
    cpack[:, 42:44] = np.stack([np.arange(128), np.arange(128, 256)], axis=1)

    epack = (np.concatenate([embed[0:128], embed[128:256]], axis=1)
             * np.float32(1.0 / ia)).astype(NP_BF16)

    wpack = np.zeros((128, 1280), np.float32)
    wpack[:, 0:256] = W1[0:128]
    wpack[:, 256:512] = W1[128:256]
    wpack[:, 512:768] = W1[256:384]
    wpack[:, 768:1024] = np.concatenate([W2[0:128], W2[128:256]], axis=1)
    ghw = head_w * ln_g[:, None]
    wpack[:, 1024:1280] = (ghw - ghw.mean(axis=0, keepdims=True)) * np.float32(ia)
    wpack = wpack.astype(NP_BF16)

    fpack = np.zeros((128, 768), np.float32)
    fpack[:, 0:256] = W1[384:512]
    fpack[:, 256:512] = g["Wc1"]
    fpack[:, 512:768] = np.concatenate([g["Wc2"][0:128], g["Wc2"][128:256]], axis=1)

    tokens = g["tokens"]
    in_maps = []
    for c in range(NC):
        tk = tokens[c * BL:(c + 1) * BL].astype(np.float32)   # (BL, T)
        in_maps.append({
            "tok": np.ascontiguousarray(tk.T).reshape(1, NTOK).astype(NP_BF16),
            "bpack": bpack, "cpack": cpack, "epack": epack,
            "wpack": wpack, "fpack": fpack,
        })

    kw = {}
    if TRACE:
        kw = dict(trace=True)
    res = run_bass_kernel_spmd(nc, in_maps, core_ids=list(range(NC)), **kw)
    if TRACE and res.exec_time_ns is not None:
        print(f"HW exec time: {res.exec_time_ns} ns")
        kernel.last_exec_ns = res.exec_time_ns
        kernel.last_trace = res.instructions_and_trace
    out = np.stack([np.asarray(res.results[c]["out"], dtype=np.float32)
                    for c in range(NC)], axis=0)
    out = out.reshape(B, T, V)
    if has_lnb:
        out = out + (ln_b @ head_w)[None, None, :]
    return np.ascontiguousarray(out)
